# revision 1
# baseline (speedup 1.0000x reference)
"""Trainium2 Bass kernel for MiniSelectiveSSM.

Reference computation (per batch row b):
    a = sigmoid(x @ Wa + ba)          # (T, N)
    u = (1 - a) * (x @ Wb + bb)       # (T, N)
    c = tanh(x @ Wc + bc)             # (T, N)
    s_t = a_t * s_{t-1} + u_t         # scan over T
    y = (c * s) @ Wy + by + x @ Wd + bd   # (T, D)

Sharding: data-parallel over batch B=8 across the 8 NeuronCores (one batch
row per core); projection weights replicated; the time scan stays local.

Layout: everything on-device is "transposed" — channels on partitions, time
on the free dimension. The host feeds x[b].T (D, T) so every GEMM contracts
over the partition dim with no on-device transposes, and the T-recurrence
maps directly onto the DVE's native tensor_tensor_scan instruction
(state = data0*state + data1 along the free dim, one recurrence per
partition).
"""

import os
import sys

import numpy as np


def _ensure_paths():
    for p in ("/opt/trn_rl_repo", "/root/.axon_site/_ro/trn_rl_repo"):
        if os.path.isdir(p) and p not in sys.path:
            sys.path.insert(0, p)


_ensure_paths()

import concourse.bass as bass  # noqa: E402
import concourse.tile as tile  # noqa: E402
from concourse import bacc, mybir  # noqa: E402
from concourse.bass_utils import run_bass_kernel_spmd  # noqa: E402

# Problem shapes (hardcoded per contract).
B, T, D, N = 8, 2048, 1024, 256
NCORES = 8
P = 128
KD = D // P   # 8  K-tiles over D
KN = N // P   # 2  K-tiles over N
TB = 512      # T-block (matmul moving free dim)
NB = T // TB  # 4 blocks

F32 = mybir.dt.float32
ALU = mybir.AluOpType
AF = mybir.ActivationFunctionType

# Matmul operand dtype: "f32" (exact, 4 cyc/row) or "f32r" (replicated fp32,
# 1 cyc/row at moving>=256, near-fp32 precision).
MM_DT = os.environ.get("SSM_MM_DT", "f32r")
PIPE = os.environ.get("SSM_PIPE", "1") == "1"


MMD = {
    "f32": F32,
    "f32r": mybir.dt.float32r,
    "bf16": mybir.dt.bfloat16,
}[MM_DT]
# DRAM dtype of matmul inputs: bf16 arrays are cast host-side.
DRAM_MM_DT = mybir.dt.bfloat16 if MM_DT == "bf16" else F32
# Blocks whose gate/output GEMMs share each stationary operand (weight-load
# amortization on the PE): consecutive matmuls differing only in the moving
# operand reuse the loaded stationary.
PAIR = int(os.environ.get("SSM_PAIR", "2"))
# y-phase stationary-sharing width (all x blocks are resident, so the
# output GEMMs can amortize each weight load over more moving blocks).
YPAIR = int(os.environ.get("SSM_YPAIR", "4"))
assert NB % PAIR == 0 and NB % YPAIR == 0


def _src(ap):
    """DRAM-side view matching the SBUF storage dtype (pure bitcast)."""
    return ap.bitcast(MMD) if MMD != ap.dtype else ap


def build_nc(reps: int = 1):
    """Build the Bass module. reps>1 wraps the pipeline in an on-device
    repeat loop (identical work each iteration) — used only for timing,
    since per-call dispatch overhead through the axon tunnel is ~ms."""
    nc = bacc.Bacc("TRN2", target_bir_lowering=False, debug=False)

    xT = nc.dram_tensor("xT", [D, T], DRAM_MM_DT, kind="ExternalInput")
    Wa = nc.dram_tensor("Wa", [D, N], DRAM_MM_DT, kind="ExternalInput")
    Wb = nc.dram_tensor("Wb", [D, N], DRAM_MM_DT, kind="ExternalInput")
    Wc = nc.dram_tensor("Wc", [D, N], DRAM_MM_DT, kind="ExternalInput")
    Wd = nc.dram_tensor("Wd", [D, D], DRAM_MM_DT, kind="ExternalInput")
    Wy = nc.dram_tensor("Wy", [N, D], DRAM_MM_DT, kind="ExternalInput")
    # Biases pre-shaped host-side to [P, groups]: col h holds bias[h*128+p].
    ba2 = nc.dram_tensor("ba2", [P, KN], F32, kind="ExternalInput")
    nba2 = nc.dram_tensor("nba2", [P, KN], F32, kind="ExternalInput")
    bb2 = nc.dram_tensor("bb2", [P, KN], F32, kind="ExternalInput")
    bc2 = nc.dram_tensor("bc2", [P, KN], F32, kind="ExternalInput")
    bY8 = nc.dram_tensor("bY8", [P, KD], F32, kind="ExternalInput")  # by + bd

    yT = nc.dram_tensor("yT", [D, T], F32, kind="ExternalOutput")

    xT_r = xT.ap().rearrange("(k p) t -> p k t", p=P)
    yT_r = yT.ap().rearrange("(m p) t -> p m t", p=P)

    with tile.TileContext(nc) as tc:
        with (
            tc.tile_pool(name="wpool", bufs=1) as wpool,
            tc.tile_pool(name="xpool", bufs=2) as xpool,
            tc.tile_pool(name="gpool", bufs=2) as gpool,
            tc.tile_pool(name="ypool", bufs=2) as ypool,
            tc.tile_pool(name="psp", bufs=8, space="PSUM") as psp,
        ):
            # ---- replicated weights / biases into SBUF (once) ----
            # Chunked per k-tile so the first gate matmuls only wait for the
            # k=0 slices, not the full 8 MB of weights. Wd/Wy (y-phase) load
            # after the gate weights.
            wa_sb = wpool.tile([P, KD, N], MMD)
            wb_sb = wpool.tile([P, KD, N], MMD)
            wc_sb = wpool.tile([P, KD, N], MMD)
            wd_sb = wpool.tile([P, KD, D], MMD)
            wy_sb = wpool.tile([P, KN, D], MMD)
            wa_r = _src(Wa.ap().rearrange("(k p) n -> p k n", p=P))
            wb_r = _src(Wb.ap().rearrange("(k p) n -> p k n", p=P))
            wc_r = _src(Wc.ap().rearrange("(k p) n -> p k n", p=P))
            wd_r = _src(Wd.ap().rearrange("(k p) n -> p k n", p=P))
            wy_r = _src(Wy.ap().rearrange("(k p) n -> p k n", p=P))
            # All first-rep x tiles preallocated so their chunks issue in
            # consumption order: the first gate matmul waits on ~1 MB, not
    # 8 MB, and pair-1's x arrives before the y-phase weights.
            x_pre = [
                xpool.tile([P, KD, TB], MMD, name=f"x_sb_{blk}",
                           tag=f"x_sb{blk % NB}", bufs=1)
                for blk in range(NB)
            ]
            for k in range(KD):
                nc.sync.dma_start(wa_sb[:, k], wa_r[:, k])
                for tb in range(PAIR):
                    nc.sync.dma_start(
                        x_pre[tb][:, k],
                        _src(xT_r[:, k, slice(tb * TB, (tb + 1) * TB)]),
                    )

            ba_sb = wpool.tile([P, KN], F32)
            nc.sync.dma_start(ba_sb[:], ba2.ap())
            nba_sb = wpool.tile([P, KN], F32)
            nc.sync.dma_start(nba_sb[:], nba2.ap())
            bb_sb = wpool.tile([P, KN], F32)
            nc.sync.dma_start(bb_sb[:], bb2.ap())
            bc_sb = wpool.tile([P, KN], F32)
            nc.sync.dma_start(bc_sb[:], bc2.ap())
            by_sb = wpool.tile([P, KD], F32)
            nc.sync.dma_start(by_sb[:], bY8.ap())

            for k in range(KD):
                nc.sync.dma_start(wb_sb[:, k], wb_r[:, k])
                nc.sync.dma_start(wc_sb[:, k], wc_r[:, k])
            for blk in range(PAIR, NB):
                for k in range(KD):
                    nc.sync.dma_start(
                        x_pre[blk][:, k],
                        _src(xT_r[:, k, slice(blk * TB, (blk + 1) * TB)]),
                    )
            for k in range(KN):
                nc.sync.dma_start(wy_sb[:, k], wy_r[:, k])
            for k in range(KD):
                nc.sync.dma_start(wd_sb[:, k], wd_r[:, k])

            def emit_gates_pair(blks, s_prev, preloaded=None):
                x_sbs, gt = [], {}
                for tb, blk in enumerate(blks):
                    tcol = slice(blk * TB, (blk + 1) * TB)
                    if preloaded is not None:
                        x_sb = preloaded[tb]
                    else:
                        x_sb = xpool.tile([P, KD, TB], MMD,
                                          name=f"x_sb_{blk}",
                                          tag=f"x_sb{blk % NB}", bufs=1)
                        for k in range(KD):
                            nc.sync.dma_start(x_sb[:, k],
                                              _src(xT_r[:, k, tcol]))
                    x_sbs.append(x_sb)
                    gbufs = {"a": 2, "am1": 2, "u": 2, "c": 2, "s": 2,
                             "cs": max(2 * PAIR, YPAIR + 2)}
                    gt[blk] = {
                        nm: gpool.tile([P, KN, TB],
                                       MMD if nm == "cs" else F32,
                                       name=f"{nm}_{blk}", tag=nm, bufs=nb)
                        for nm, nb in gbufs.items()
                    }

                # ---- gate GEMMs: zA/zB first (scan inputs), zC after the
                # scan is already running on DVE ----
                for wsb, kind in ((wa_sb, "a"), (wb_sb, "b")):
                    for m in range(KN):
                        mcol = slice(m * P, (m + 1) * P)
                        pss = [
                            psp.tile([P, TB], F32,
                                     name=f"ps_{kind}{m}_{blk}", tag="ps")
                            for blk in blks
                        ]
                        for k in range(KD):
                            for tb in range(len(blks)):
                                nc.tensor.matmul(
                                    pss[tb][:],
                                    wsb[:, k, mcol],
                                    x_sbs[tb][:, k, :],
                                    start=(k == 0),
                                    stop=(k == KD - 1),
                                )
                        for tb, blk in enumerate(blks):
                            g = gt[blk]
                            ps = pss[tb]
                            if kind == "a":
                                nc.scalar.activation(
                                    g["a"][:, m, :], ps[:], AF.Sigmoid,
                                    bias=ba_sb[:, m : m + 1], scale=1.0,
                                )
                                nc.scalar.activation(
                                    g["am1"][:, m, :], ps[:], AF.Sigmoid,
                                    bias=nba_sb[:, m : m + 1], scale=-1.0,
                                )
                            elif kind == "b":
                                # u = (zB + bb) * (1 - a), from PSUM on DVE
                                nc.vector.scalar_tensor_tensor(
                                    g["u"][:, m, :], ps[:], bb_sb[:, m : m + 1],
                                    g["am1"][:, m, :], op0=ALU.add, op1=ALU.mult,
                                )
                # ---- time recurrence: one native scan per N-half ----
                for blk in blks:
                    g = gt[blk]
                    for m in range(KN):
                        init = (
                            0.0 if s_prev is None
                            else s_prev[:, m, TB - 1 : TB]
                        )
                        nc.vector.tensor_tensor_scan(
                            g["s"][:, m, :], g["a"][:, m, :], g["u"][:, m, :],
                            init, op0=ALU.mult, op1=ALU.add,
                        )
                    s_prev = g["s"]

                # ---- c-gate GEMMs overlap the scan; cs right after ----
                for m in range(KN):
                    mcol = slice(m * P, (m + 1) * P)
                    pss = [
                        psp.tile([P, TB], F32,
                                 name=f"ps_c{m}_{blk}", tag="ps")
                        for blk in blks
                    ]
                    for k in range(KD):
                        for tb in range(len(blks)):
                            nc.tensor.matmul(
                                pss[tb][:],
                                wc_sb[:, k, mcol],
                                x_sbs[tb][:, k, :],
                                start=(k == 0),
                                stop=(k == KD - 1),
                            )
                    for tb, blk in enumerate(blks):
                        g = gt[blk]
                        nc.scalar.activation(
                            g["c"][:, m, :], pss[tb][:], AF.Tanh,
                            bias=bc_sb[:, m : m + 1], scale=1.0,
                        )
                        nc.vector.tensor_tensor(
                            g["cs"][:, m, :], g["c"][:, m, :],
                            g["s"][:, m, :], ALU.mult,
                        )
                cs_ts = [gt[blk]["cs"] for blk in blks]
                return x_sbs, cs_ts, s_prev

            def emit_y_pair(blks, x_sbs, cs_ts):
                # ---- output GEMM: yT = Wd.T@xT + Wy.T@cs (+ by+bd) ----
                for m in range(KD):
                    mcol = slice(m * P, (m + 1) * P)
                    pss = [
                        psp.tile([P, TB], F32, name=f"ps_y{m}_{blk}",
                                 tag="ps")
                        for blk in blks
                    ]
                    for k in range(KD):
                        for tb in range(len(blks)):
                            nc.tensor.matmul(
                                pss[tb][:],
                                wd_sb[:, k, mcol],
                                x_sbs[tb][:, k, :],
                                start=(k == 0),
                                stop=False,
                            )
                    for k in range(KN):
                        for tb in range(len(blks)):
                            nc.tensor.matmul(
                                pss[tb][:],
                                wy_sb[:, k, mcol],
                                cs_ts[tb][:, k, :],
                                start=False,
                                stop=(k == KN - 1),
                            )
                    for tb, blk in enumerate(blks):
                        tcol = slice(blk * TB, (blk + 1) * TB)
                        ym = ypool.tile([P, TB], F32, name=f"ym_{m}_{blk}",
                                        tag="ym", bufs=6)
                        nc.scalar.activation(
                            ym[:], pss[tb][:], AF.Identity,
                            bias=by_sb[:, m : m + 1], scale=1.0,
                        )
                        nc.sync.dma_start(yT_r[:, m, tcol], ym[:])

            def emit_body():
                # Gates run in PAIR-wide passes; y-GEMMs run in YPAIR-wide
                # passes emitted one pass late (PIPE) so PE never stalls
                # waiting for the scan.
                s_prev = None
                pending = []
                ready = []  # (blk, x_sb, cs_t) with gates emitted
                for p0 in range(0, NB, PAIR):
                    blks = list(range(p0, p0 + PAIR))
                    pre = (x_pre[p0 : p0 + PAIR]
                           if emit_body.first else None)
                    x_sbs, cs_ts, s_prev = emit_gates_pair(blks, s_prev, pre)
                    ready.extend(zip(blks, x_sbs, cs_ts))
                    if len(ready) == YPAIR:
                        grp = (
                            [r[0] for r in ready],
                            [r[1] for r in ready],
                            [r[2] for r in ready],
                        )
                        ready = []
                        if PIPE:
                            pending.append(grp)
                            if len(pending) > 1:
                                emit_y_pair(*pending.pop(0))
                        else:
                            emit_y_pair(*grp)
                for grp in pending:
                    emit_y_pair(*grp)

            # Static unroll for timing builds: dynamic For_i loops measured
            # ~40 ms/iteration under this axon runtime, so they're unusable.
            emit_body.first = True
            for _ in range(reps):
                emit_body()
                emit_body.first = False

    nc.compile()
    return nc


_NC_CACHE = {}


def _get_nc():
    key = MM_DT
    if key not in _NC_CACHE:
        _NC_CACHE[key] = build_nc()
    return _NC_CACHE[key]


def make_in_maps(x, Wa, ba, Wb, bb, Wc, bc, Wd, bd, Wy, by):
    x = np.asarray(x, np.float32)
    f = np.float32
    ba2 = np.ascontiguousarray(np.asarray(ba, f).reshape(KN, P).T)
    nba2 = np.ascontiguousarray(-np.asarray(ba, f).reshape(KN, P).T)
    bb2 = np.ascontiguousarray(np.asarray(bb, f).reshape(KN, P).T)
    bc2 = np.ascontiguousarray(np.asarray(bc, f).reshape(KN, P).T)
    bY8 = np.ascontiguousarray(
        (np.asarray(by, f) + np.asarray(bd, f)).reshape(KD, P).T
    )
    shared = {
        "Wa": np.ascontiguousarray(np.asarray(Wa, f)),
        "Wb": np.ascontiguousarray(np.asarray(Wb, f)),
        "Wc": np.ascontiguousarray(np.asarray(Wc, f)),
        "Wd": np.ascontiguousarray(np.asarray(Wd, f)),
        "Wy": np.ascontiguousarray(np.asarray(Wy, f)),
        "ba2": ba2, "nba2": nba2, "bb2": bb2, "bc2": bc2, "bY8": bY8,
    }
    return [
        {"xT": np.ascontiguousarray(x[b].T), **shared} for b in range(NCORES)
    ]


def kernel(x, Wa, ba, Wb, bb, Wc, bc, Wd, bd, Wy, by):
    in_maps = make_in_maps(x, Wa, ba, Wb, bb, Wc, bc, Wd, bd, Wy, by)
    last_err = None
    for attempt in range(3):
        try:
            nc = _get_nc()
            res = run_bass_kernel_spmd(nc, in_maps,
                                       core_ids=list(range(NCORES)))
            break
        except Exception as e:  # transient NRT device faults happen
            last_err = e
            _NC_CACHE.clear()
            import time as _time

            _time.sleep(2.0 * (attempt + 1))
    else:
        raise last_err
    y = np.stack([res.results[b]["yT"].T for b in range(NCORES)], axis=0)
    return np.ascontiguousarray(y.astype(np.float32))


if __name__ == "__main__":
    rng = np.random.default_rng(0)
    sD = 1.0 / np.sqrt(D)
    sN = 1.0 / np.sqrt(N)
    inputs = {
        "x": rng.standard_normal((B, T, D), dtype=np.float32),
        "Wa": rng.standard_normal((D, N), dtype=np.float32) * sD,
        "ba": np.zeros(N, np.float32),
        "Wb": rng.standard_normal((D, N), dtype=np.float32) * sD,
        "bb": np.zeros(N, np.float32),
        "Wc": rng.standard_normal((D, N), dtype=np.float32) * sD,
        "bc": np.zeros(N, np.float32),
        "Wd": rng.standard_normal((D, D), dtype=np.float32) * sD,
        "bd": np.zeros(D, np.float32),
        "Wy": rng.standard_normal((N, D), dtype=np.float32) * sN,
        "by": np.zeros(D, np.float32),
    }
    y = kernel(**inputs)
    print("y", y.shape, y.dtype, float(np.abs(y).max()))



# revision 2
# speedup vs baseline: 373.3912x; 373.3912x over previous
"""Trainium2 Bass kernel for MiniSelectiveSSM.

Reference computation (per batch row b):
    a = sigmoid(x @ Wa + ba)          # (T, N)
    u = (1 - a) * (x @ Wb + bb)       # (T, N)
    c = tanh(x @ Wc + bc)             # (T, N)
    s_t = a_t * s_{t-1} + u_t         # scan over T
    y = (c * s) @ Wy + by + x @ Wd + bd   # (T, D)

Sharding: data-parallel over batch B=8 across the 8 NeuronCores (one batch
row per core); projection weights replicated; the time scan stays local.

Layout: channels on partitions, time on the free dimension; the host feeds
x[b].T pre-arranged as [P, NB, KD, TB] so each T-block loads with a single
contiguous DMA, and the T-recurrence maps onto the DVE's native
tensor_tensor_scan.

Precision strategy (validated against the reference inputs in numpy):
  - za, zb GEMMs: fp8 e4m3 with DoubleRow perf mode (2 K-subtiles per
    PE pass -> ~2x MAC rate). x and Wa/Wb are pre-scaled host-side
    (sx=32, sw=pick_scale) and clipped to +-240. The sigmoid activation
    descales via its `scale` argument; the zb path stays scaled through
    the (linear) scan and is compensated by scaling Wy down host-side.
  - zc, d, y GEMMs: bf16 (same PE rate as f32r, half the DMA traffic).
  - PSUM accumulation, scan, activations: fp32.  y output: bf16.
Measured end-to-end rel err vs the fp32 reference: ~1.6e-2 (< 2e-2).
"""

import os
import sys

import numpy as np
import ml_dtypes


def _ensure_paths():
    for p in ("/opt/trn_rl_repo", "/root/.axon_site/_ro/trn_rl_repo"):
        if os.path.isdir(p) and p not in sys.path:
            sys.path.insert(0, p)


_ensure_paths()

import concourse.bass as bass  # noqa: E402,F401
import concourse.tile as tile  # noqa: E402
from concourse import bacc, mybir  # noqa: E402
from concourse.bass_utils import run_bass_kernel_spmd  # noqa: E402

# Problem shapes (hardcoded per contract).
B, T, D, N = 8, 2048, 1024, 256
NCORES = 8
P = 128
KD = D // P   # 8  K-tiles over D
KN = N // P   # 2  K-tiles over N
TB = 512      # T-block (matmul moving free dim)
NB = T // TB  # 4 blocks

F32 = mybir.dt.float32
BF16 = mybir.dt.bfloat16
FP8 = mybir.dt.float8e4
ALU = mybir.AluOpType
AF = mybir.ActivationFunctionType
DR = mybir.MatmulPerfMode.DoubleRow

# Which gate GEMMs run in fp8 DoubleRow ("a","b") and base dtype for the
# rest ("bf16" or "f32r"). Both validated within tolerance in prec sims.
FP8_GATES = tuple(os.environ.get("SSM_FP8", "ab"))
MM_DT = os.environ.get("SSM_MM_DT", "bf16")
MMD = {"bf16": BF16, "f32r": mybir.dt.float32r}[MM_DT]
NPDT = {"bf16": ml_dtypes.bfloat16, "f32r": np.float32}[MM_DT]
YOUT = os.environ.get("SSM_YOUT", "bf16")
YDT = BF16 if YOUT == "bf16" else F32
YNP = ml_dtypes.bfloat16 if YOUT == "bf16" else np.float32
PAIR = int(os.environ.get("SSM_PAIR", "2"))
PIPE = os.environ.get("SSM_PIPE", "1") == "1"
SX = 32.0  # fp8 pre-scale on x


def build_nc(reps: int = 1, s_a: float = 32768.0, s_b: float = 32768.0):
    """Build the Bass module. reps>1 wraps the pipeline in a statically
    unrolled repeat loop (identical work each iteration) — used only for
    timing, since per-call dispatch through the axon tunnel is ~ms."""
    nc = bacc.Bacc("TRN2", target_bir_lowering=False, debug=False)

    fp8_a = "a" in FP8_GATES
    fp8_b = "b" in FP8_GATES

    # Inputs, all pre-arranged host-side (see make_in_maps).
    xq8 = nc.dram_tensor("xq8", [P, NB, KD, TB], FP8, kind="ExternalInput")
    x16 = nc.dram_tensor("x16", [P, NB, KD, TB], MMD, kind="ExternalInput")
    Wa = nc.dram_tensor("Wa", [P, KD, N], FP8 if fp8_a else MMD,
                        kind="ExternalInput")
    Wb = nc.dram_tensor("Wb", [P, KD, N], FP8 if fp8_b else MMD,
                        kind="ExternalInput")
    Wc = nc.dram_tensor("Wc", [P, KD, N], MMD, kind="ExternalInput")
    Wd = nc.dram_tensor("Wd", [P, KD, D], MMD, kind="ExternalInput")
    Wy = nc.dram_tensor("Wy", [P, KN, D], MMD, kind="ExternalInput")
    # Biases pre-shaped host-side to [P, groups]: col h holds bias[h*128+p].
    ba2 = nc.dram_tensor("ba2", [P, KN], F32, kind="ExternalInput")
    nba2 = nc.dram_tensor("nba2", [P, KN], F32, kind="ExternalInput")
    bb2 = nc.dram_tensor("bb2", [P, KN], F32, kind="ExternalInput")
    bc2 = nc.dram_tensor("bc2", [P, KN], F32, kind="ExternalInput")
    bY8 = nc.dram_tensor("bY8", [P, KD], F32, kind="ExternalInput")  # by+bd

    yP = nc.dram_tensor("yP", [P, NB, KD, TB], YDT, kind="ExternalOutput")

    with tile.TileContext(nc) as tc:
        with (
            tc.tile_pool(name="wpool", bufs=1) as wpool,
            tc.tile_pool(name="xpool", bufs=2) as xpool,
            tc.tile_pool(name="gpool", bufs=2) as gpool,
            tc.tile_pool(name="ypool", bufs=2) as ypool,
            tc.tile_pool(name="psp", bufs=8, space="PSUM") as psp,
        ):
            # ---- replicated weights / biases into SBUF (once) ----
            wa_sb = wpool.tile([P, KD, N], FP8 if fp8_a else MMD)
            wb_sb = wpool.tile([P, KD, N], FP8 if fp8_b else MMD)
            nc.sync.dma_start(wa_sb[:], Wa.ap())
            nc.sync.dma_start(wb_sb[:], Wb.ap())
            ba_sb = wpool.tile([P, KN], F32)
            nc.sync.dma_start(ba_sb[:], ba2.ap())
            nba_sb = wpool.tile([P, KN], F32)
            nc.sync.dma_start(nba_sb[:], nba2.ap())
            bb_sb = wpool.tile([P, KN], F32)
            nc.sync.dma_start(bb_sb[:], bb2.ap())
            bc_sb = wpool.tile([P, KN], F32)
            nc.sync.dma_start(bc_sb[:], bc2.ap())
            by_sb = wpool.tile([P, KD], F32)
            nc.sync.dma_start(by_sb[:], bY8.ap())
            wc_sb = wpool.tile([P, KD, N], MMD)
            nc.sync.dma_start(wc_sb[:], Wc.ap())
            wy_sb = wpool.tile([P, KN, D], MMD)
            nc.sync.dma_start(wy_sb[:], Wy.ap())
            wd_sb = wpool.tile([P, KD, D], MMD)
            for k in range(KD):
                nc.sync.dma_start(wd_sb[:, k], Wd.ap()[:, k])

            def gate_matmuls(wsb, fp8, m, pss, x_fp8s, x_16s):
                """One m-tile of a gate GEMM over all blocks in the pair."""
                mcol = slice(m * P, (m + 1) * P)
                if fp8:
                    for kp in range(KD // 2):
                        ks = slice(2 * kp, 2 * kp + 2)
                        for tb in range(len(pss)):
                            nc.tensor.matmul(
                                pss[tb][:], wsb[:, ks, mcol],
                                x_fp8s[tb][:, ks, :],
                                start=(kp == 0), stop=(kp == KD // 2 - 1),
                                perf_mode=DR,
                            )
                else:
                    for k in range(KD):
                        for tb in range(len(pss)):
                            nc.tensor.matmul(
                                pss[tb][:], wsb[:, k, mcol],
                                x_16s[tb][:, k, :],
                                start=(k == 0), stop=(k == KD - 1),
                            )

            def emit_gates_pair(blks, s_prev):
                xq_sbs, x16_sbs, gt = [], [], {}
                for tb, blk in enumerate(blks):
                    xq_sb = xpool.tile([P, KD, TB], FP8, name=f"xq_{blk}",
                                       tag=f"xq{blk % PAIR}", bufs=2)
                    nc.sync.dma_start(xq_sb[:], xq8.ap()[:, blk])
                    x16_sb = xpool.tile([P, KD, TB], MMD, name=f"x16_{blk}",
                                        tag=f"x16_{blk % NB}", bufs=1)
                    nc.sync.dma_start(x16_sb[:], x16.ap()[:, blk])
                    xq_sbs.append(xq_sb)
                    x16_sbs.append(x16_sb)
                    gbufs = {"a": 2, "am1": 2, "u": 2, "c": 2, "s": 2,
                             "cs": 2 * PAIR + 2}
                    gt[blk] = {
                        nm: gpool.tile([P, KN, TB], MMD if nm == "cs" else F32,
                                       name=f"{nm}_{blk}", tag=nm, bufs=nb)
                        for nm, nb in gbufs.items()
                    }

                # ---- gate GEMMs: zA/zB first (scan inputs) ----
                for wsb, fp8, kind in ((wa_sb, fp8_a, "a"),
                                       (wb_sb, fp8_b, "b")):
                    inv = (1.0 / (s_a if kind == "a" else s_b)) if fp8 else 1.0
                    for m in range(KN):
                        pss = [
                            psp.tile([P, TB], F32,
                                     name=f"ps_{kind}{m}_{blk}", tag="ps")
                            for blk in blks
                        ]
                        gate_matmuls(wsb, fp8, m, pss, xq_sbs, x16_sbs)
                        for tb, blk in enumerate(blks):
                            g = gt[blk]
                            ps = pss[tb]
                            if kind == "a":
                                nc.scalar.activation(
                                    g["a"][:, m, :], ps[:], AF.Sigmoid,
                                    bias=ba_sb[:, m : m + 1], scale=inv,
                                )
                                nc.scalar.activation(
                                    g["am1"][:, m, :], ps[:], AF.Sigmoid,
                                    bias=nba_sb[:, m : m + 1], scale=-inv,
                                )
                            else:
                                # u' = (zb' + bb*s_b) * (1 - a); the s_b
                                # scale rides through the linear scan and is
                                # compensated in Wy host-side.
                                nc.vector.scalar_tensor_tensor(
                                    g["u"][:, m, :], ps[:],
                                    bb_sb[:, m : m + 1],
                                    g["am1"][:, m, :],
                                    op0=ALU.add, op1=ALU.mult,
                                )
                # ---- time recurrence: one native scan per N-half ----
                for blk in blks:
                    g = gt[blk]
                    for m in range(KN):
                        init = (
                            0.0 if s_prev is None
                            else s_prev[:, m, TB - 1 : TB]
                        )
                        nc.vector.tensor_tensor_scan(
                            g["s"][:, m, :], g["a"][:, m, :], g["u"][:, m, :],
                            init, op0=ALU.mult, op1=ALU.add,
                        )
                    s_prev = g["s"]

                # ---- c-gate GEMMs (bf16) overlap the scan; cs after ----
                for m in range(KN):
                    mcol = slice(m * P, (m + 1) * P)
                    pss = [
                        psp.tile([P, TB], F32,
                                 name=f"ps_c{m}_{blk}", tag="ps")
                        for blk in blks
                    ]
                    for k in range(KD):
                        for tb in range(len(blks)):
                            nc.tensor.matmul(
                                pss[tb][:], wc_sb[:, k, mcol],
                                x16_sbs[tb][:, k, :],
                                start=(k == 0), stop=(k == KD - 1),
                            )
                    for tb, blk in enumerate(blks):
                        g = gt[blk]
                        nc.scalar.activation(
                            g["c"][:, m, :], pss[tb][:], AF.Tanh,
                            bias=bc_sb[:, m : m + 1], scale=1.0,
                        )
                        nc.vector.tensor_tensor(
                            g["cs"][:, m, :], g["c"][:, m, :],
                            g["s"][:, m, :], ALU.mult,
                        )
                cs_ts = [gt[blk]["cs"] for blk in blks]
                return x16_sbs, cs_ts, s_prev

            def emit_y_pair(blks, x_sbs, cs_ts):
                # ---- output GEMM: yT = Wd.T@x + Wy.T@cs (+ by+bd) ----
                yms = [
                    ypool.tile([P, KD, TB], YDT, name=f"ym_{blk}",
                               tag="ym", bufs=4)
                    for blk in blks
                ]
                for m in range(KD):
                    mcol = slice(m * P, (m + 1) * P)
                    pss = [
                        psp.tile([P, TB], F32, name=f"ps_y{m}_{blk}",
                                 tag="ps")
                        for blk in blks
                    ]
                    for k in range(KD):
                        for tb in range(len(blks)):
                            nc.tensor.matmul(
                                pss[tb][:], wd_sb[:, k, mcol],
                                x_sbs[tb][:, k, :],
                                start=(k == 0), stop=False,
                            )
                    for k in range(KN):
                        for tb in range(len(blks)):
                            nc.tensor.matmul(
                                pss[tb][:], wy_sb[:, k, mcol],
                                cs_ts[tb][:, k, :],
                                start=False, stop=(k == KN - 1),
                            )
                    for tb in range(len(blks)):
                        nc.scalar.activation(
                            yms[tb][:, m, :], pss[tb][:], AF.Identity,
                            bias=by_sb[:, m : m + 1], scale=1.0,
                        )
                for tb, blk in enumerate(blks):
                    nc.sync.dma_start(yP.ap()[:, blk], yms[tb][:])

            def emit_body():
                # Gates run in PAIR-wide passes; each pair's y-GEMMs are
                # emitted one pair late (PIPE) so PE never stalls on the
                # scan. One-pair delay also keeps the x16/cs buffer reuse
                # across reps correctly ordered.
                s_prev = None
                pending = []
                for p0 in range(0, NB, PAIR):
                    blks = list(range(p0, p0 + PAIR))
                    x_sbs, cs_ts, s_prev = emit_gates_pair(blks, s_prev)
                    grp = (blks, x_sbs, cs_ts)
                    if PIPE:
                        pending.append(grp)
                        if len(pending) > 1:
                            emit_y_pair(*pending.pop(0))
                    else:
                        emit_y_pair(*grp)
                return pending

            pending = []
            for _ in range(reps):
                for grp in pending:
                    emit_y_pair(*grp)
                pending = emit_body()
            for grp in pending:
                emit_y_pair(*grp)

    nc.compile()
    return nc


_NC_CACHE = {}


def _pick_scale(v):
    m = float(np.abs(v).max())
    s = 1.0
    while m * s * 2 <= 240.0:
        s *= 2
    while m * s > 240.0 and s > 2 ** -40:
        s /= 2
    return s


def _q8(v, scale):
    return np.clip(v * scale, -240.0, 240.0).astype(ml_dtypes.float8_e4m3fn)


def make_in_maps(x, Wa, ba, Wb, bb, Wc, bc, Wd, bd, Wy, by):
    f = np.float32
    x = np.asarray(x, f)
    Wa, Wb = np.asarray(Wa, f), np.asarray(Wb, f)
    Wc, Wd, Wy = np.asarray(Wc, f), np.asarray(Wd, f), np.asarray(Wy, f)
    ba, bb = np.asarray(ba, f), np.asarray(bb, f)
    bc, bd, by = np.asarray(bc, f), np.asarray(bd, f), np.asarray(by, f)

    fp8_a = "a" in FP8_GATES
    fp8_b = "b" in FP8_GATES
    swa = _pick_scale(Wa) if fp8_a else 1.0
    swb = _pick_scale(Wb) if fp8_b else 1.0
    s_a = SX * swa
    s_b = SX * swb

    def warr(w, n_groups, quant, scale):
        # [D, n] -> [P, k, n]
        w3 = np.ascontiguousarray(
            w.reshape(n_groups, P, w.shape[1]).transpose(1, 0, 2))
        if quant:
            return _q8(w3, scale)
        return np.ascontiguousarray(w3.astype(NPDT))

    ba2 = np.ascontiguousarray(ba.reshape(KN, P).T)
    nba2 = np.ascontiguousarray(-ba.reshape(KN, P).T)
    # bb rides at the zb' scale (compensated via Wy).
    bb2 = np.ascontiguousarray((bb * s_b).reshape(KN, P).T)
    bc2 = np.ascontiguousarray(bc.reshape(KN, P).T)
    bY8 = np.ascontiguousarray((by + bd).reshape(KD, P).T)

    shared = {
        "Wa": warr(Wa, KD, fp8_a, swa),
        "Wb": warr(Wb, KD, fp8_b, swb),
        "Wc": warr(Wc, KD, False, 1.0),
        "Wd": warr(Wd, KD, False, 1.0),
        "Wy": warr(Wy / s_b, KN, False, 1.0),
        "ba2": ba2, "nba2": nba2, "bb2": bb2, "bc2": bc2, "bY8": bY8,
    }

    in_maps = []
    for b in range(NCORES):
        # x[b].T as [P, NB, KD, TB]: xT[(k p), (nb tb)] -> [p, nb, k, tb]
        xT = x[b].T.reshape(KD, P, NB, TB).transpose(1, 2, 0, 3)
        xq = _q8(xT, SX)
        x16b = np.ascontiguousarray(xT.astype(NPDT))
        in_maps.append({"xq8": np.ascontiguousarray(xq), "x16": x16b,
                        **shared})
    return in_maps, s_a, s_b


def kernel(x, Wa, ba, Wb, bb, Wc, bc, Wd, bd, Wy, by):
    in_maps, s_a, s_b = make_in_maps(x, Wa, ba, Wb, bb, Wc, bc, Wd, bd,
                                     Wy, by)
    key = (s_a, s_b)
    last_err = None
    for attempt in range(3):
        try:
            if key not in _NC_CACHE:
                _NC_CACHE[key] = build_nc(reps=1, s_a=s_a, s_b=s_b)
            nc = _NC_CACHE[key]
            res = run_bass_kernel_spmd(nc, in_maps,
                                       core_ids=list(range(NCORES)))
            break
        except Exception as e:  # transient NRT device faults happen
            last_err = e
            _NC_CACHE.clear()
            import time as _time

            _time.sleep(2.0 * (attempt + 1))
    else:
        raise last_err
    outs = []
    for b in range(NCORES):
        yp = np.asarray(res.results[b]["yP"]).astype(np.float32)
        # [P, NB, KD, TB] -> [T, D]
        outs.append(yp.transpose(1, 3, 2, 0).reshape(T, D))
    return np.ascontiguousarray(np.stack(outs, axis=0))


if __name__ == "__main__":
    rng = np.random.default_rng(0)
    sD = 1.0 / np.sqrt(D)
    sN = 1.0 / np.sqrt(N)
    inputs = {
        "x": rng.standard_normal((B, T, D), dtype=np.float32),
        "Wa": rng.standard_normal((D, N), dtype=np.float32) * sD,
        "ba": np.zeros(N, np.float32),
        "Wb": rng.standard_normal((D, N), dtype=np.float32) * sD,
        "bb": np.zeros(N, np.float32),
        "Wc": rng.standard_normal((D, N), dtype=np.float32) * sD,
        "bc": np.zeros(N, np.float32),
        "Wd": rng.standard_normal((D, D), dtype=np.float32) * sD,
        "bd": np.zeros(D, np.float32),
        "Wy": rng.standard_normal((N, D), dtype=np.float32) * sN,
        "by": np.zeros(D, np.float32),
    }
    y = kernel(**inputs)
    print("y", y.shape, y.dtype, float(np.abs(y).max()))
